# revision 1
# baseline (speedup 1.0000x reference)
"""Trainium2 Bass kernel for nn_FFNwMoE (MoE FFN with top-2 routing + shared expert).

Strategy (expert-parallel sparse dispatch, host-side routing):
  - Host computes router logits/softmax/top-2 (jax on CPU, bit-matching the
    reference) plus the aux load-balancing loss.
  - Tokens are gathered per expert on the host. Core e processes expert e's
    tokens (padded to capacity CA) with expert-e weights, plus a static 1/8
    slice of all tokens (CB=1024) with the shared-expert weights.
  - On-device per core: swiglu via fp32r matmuls (full PE rate, ~FP22
    precision): aT/bT = W1/W3 contraction over d, h = silu(a)*b,
    y = hT.T @ W2T accumulated over h-tiles, scaled by the combine weight.
  - Host scatter-adds the per-core outputs back into the full [T, D] output.

All heavy FLOPs (3 matmuls x (2*T top-2 assignments + T shared)) run on the
8 NeuronCores; the host only does O(T*E) routing math and data movement.
"""
import sys

if '/opt/trn_rl_repo' not in sys.path:
    sys.path.insert(0, '/opt/trn_rl_repo')

from contextlib import ExitStack

import numpy as np

import concourse.bass as bass  # noqa: F401  (bass types used via tile/bacc)
import concourse.mybir as mybir
import concourse.tile as tile
from concourse import bacc
from concourse.bass_utils import run_bass_kernel_spmd

F32R = mybir.dt.float32r
F32 = mybir.dt.float32
AF = mybir.ActivationFunctionType

# Problem constants (hardcoded per spec nn_FFNwMoE_74380243632567)
B, S, D = 4, 2048, 2048
E, TOPK, H, SHARED = 8, 2, 1368, 1
AUX_COEF = 0.01
T = B * S                      # 8192 tokens
ND = D // 128                  # 16 d-tiles
NH = (H + 127) // 128          # 11 h-tiles (H padded 1368 -> 1408)
HPAD = NH * 128
NCORES = 8
CB = T // NCORES               # shared-expert slice per core
TBMAX = 1024


def _build_moe_kernel(CA, CB):
    """One SPMD Bass program; per-core data arrives via in_maps."""
    CTOT = CA + CB
    NDC = (D + 511) // 512
    nc = bacc.Bacc("TRN2", target_bir_lowering=False, debug=False,
                   num_devices=NCORES)

    xt = nc.dram_tensor("xt", [128, ND, CTOT], F32R, kind="ExternalInput").ap()
    w13A = nc.dram_tensor("w13A", [NH, 128, ND, 256], F32R, kind="ExternalInput").ap()
    w2A = nc.dram_tensor("w2A", [NH * 128, D], F32R, kind="ExternalInput").ap()
    w13B = nc.dram_tensor("w13B", [NH, 128, ND, 256], F32R, kind="ExternalInput").ap()
    w2B = nc.dram_tensor("w2B", [NH * 128, D], F32R, kind="ExternalInput").ap()
    cw = nc.dram_tensor("cw", [CTOT, 1], F32, kind="ExternalInput").ap()
    y = nc.dram_tensor("y", [CTOT, D], F32, kind="ExternalOutput").ap()

    blocks = []
    for ph, (start, size) in enumerate(((0, CA), (CA, CB))):
        off = start
        while off < start + size:
            TB = min(TBMAX, start + size - off)
            blocks.append((off, TB, ph))
            off += TB

    with tile.TileContext(nc) as tc, ExitStack() as ctx:
        xt_pool = ctx.enter_context(tc.tile_pool(name="xtp", bufs=1))
        h_pool = ctx.enter_context(tc.tile_pool(name="hp", bufs=1))
        w13_pool = ctx.enter_context(tc.tile_pool(name="w13p", bufs=2))
        w2_pool = ctx.enter_context(tc.tile_pool(name="w2p", bufs=2))
        sil_pool = ctx.enter_context(tc.tile_pool(name="silp", bufs=2))
        y_pool = ctx.enter_context(tc.tile_pool(name="yp", bufs=3))
        cw_pool = ctx.enter_context(tc.tile_pool(name="cwp", bufs=2))
        psa_pool = ctx.enter_context(tc.tile_pool(name="psa", bufs=3, space="PSUM"))
        psb_pool = ctx.enter_context(tc.tile_pool(name="psb", bufs=3, space="PSUM"))
        psy_pool = ctx.enter_context(tc.tile_pool(name="psy", bufs=2, space="PSUM"))

        w2v_A = w2A.rearrange("(hb p) d -> p hb d", p=128)
        w2v_B = w2B.rearrange("(hb p) d -> p hb d", p=128)

        for (t0, TB, ph) in blocks:
            w13X = w13A if ph == 0 else w13B
            w2vX = w2v_A if ph == 0 else w2v_B
            ntt = TB // 128
            subs = []
            off = 0
            while off < TB:
                subs.append((off, min(512, TB - off)))
                off += 512

            xt_sb = xt_pool.tile([128, ND, TBMAX], F32R, tag="xt")
            nc.sync.dma_start(xt_sb[:, :, :TB], xt[:, :, t0:t0 + TB])

            cw_sb = cw_pool.tile([128, TBMAX // 128], F32, tag="cw")
            nc.sync.dma_start(
                cw_sb[:, :ntt],
                cw[t0:t0 + TB, 0].rearrange("(tt p) -> p tt", p=128))

            h_sb = h_pool.tile([128, NH, TBMAX], F32R, tag="hsb")

            # phase 1: aT/bT[h, t] accumulation over d-tiles; h = silu(a)*b
            for h in range(NH):
                w13_sb = w13_pool.tile([128, ND, 256], F32R, tag="w13")
                nc.sync.dma_start(w13_sb[:], w13X[h])
                for (so, sw) in subs:
                    psa = psa_pool.tile([128, 512], F32, tag="psa")
                    psb = psb_pool.tile([128, 512], F32, tag="psb")
                    for kd in range(ND):
                        nc.tensor.matmul(psa[:, :sw], w13_sb[:, kd, 0:128],
                                         xt_sb[:, kd, so:so + sw],
                                         start=(kd == 0), stop=(kd == ND - 1))
                    for kd in range(ND):
                        nc.tensor.matmul(psb[:, :sw], w13_sb[:, kd, 128:256],
                                         xt_sb[:, kd, so:so + sw],
                                         start=(kd == 0), stop=(kd == ND - 1))
                    sil = sil_pool.tile([128, 512], F32, tag="sil")
                    nc.scalar.activation(sil[:, :sw], psa[:, :sw], AF.Sigmoid)
                    nc.vector.tensor_mul(sil[:, :sw], sil[:, :sw], psa[:, :sw])
                    nc.vector.tensor_mul(h_sb[:, h, so:so + sw], sil[:, :sw],
                                         psb[:, :sw])

            # phase 3: y[t, d] = hT.T @ w2T over h-tiles, scaled by cw[t]
            for dc in range(NDC):
                dw = min(512, D - dc * 512)
                w2_sb = w2_pool.tile([128, NH, 512], F32R, tag="w2")
                nc.sync.dma_start(w2_sb[:, :, :dw],
                                  w2vX[:, :, dc * 512:dc * 512 + dw])
                for tt in range(ntt):
                    psy = psy_pool.tile([128, 512], F32, tag="psy")
                    for h in range(NH):
                        nc.tensor.matmul(psy[:, :dw],
                                         h_sb[:, h, tt * 128:(tt + 1) * 128],
                                         w2_sb[:, h, :dw],
                                         start=(h == 0), stop=(h == NH - 1))
                    y_sb = y_pool.tile([128, 512], F32, tag="y")
                    nc.vector.tensor_scalar_mul(y_sb[:, :dw], psy[:, :dw],
                                                cw_sb[:, tt:tt + 1])
                    nc.sync.dma_start(
                        y[t0 + tt * 128:t0 + (tt + 1) * 128,
                          dc * 512:dc * 512 + dw], y_sb[:, :dw])

    nc.compile()
    return nc


_KERNEL_CACHE = {}


def _get_kernel(CA):
    key = (CA, CB)
    if key not in _KERNEL_CACHE:
        _KERNEL_CACHE[key] = _build_moe_kernel(CA, CB)
    return _KERNEL_CACHE[key]


# ---------------- host-side packing ----------------

def _pack_w13(w1, w3):
    """w1, w3: [H, D] -> [NH, 128, ND, 256] with w13[h,p,kd,j]=w1p[h*128+j, kd*128+p]."""
    w1p = np.zeros((HPAD, D), np.float32)
    w1p[:H] = w1
    w3p = np.zeros((HPAD, D), np.float32)
    w3p[:H] = w3
    a = w1p.reshape(NH, 128, ND, 128).transpose(0, 3, 2, 1)
    b = w3p.reshape(NH, 128, ND, 128).transpose(0, 3, 2, 1)
    return np.ascontiguousarray(np.concatenate([a, b], axis=3))


def _pack_w2(w2):
    """w2: [D, H] -> w2T padded [NH*128, D]."""
    out = np.zeros((HPAD, D), np.float32)
    out[:H] = np.ascontiguousarray(w2.T)
    return out


def _pack_xt(xcols):
    """xcols: [CTOT, D] -> [128, ND, CTOT] with xt[p, kd, t] = xcols[t, kd*128+p]."""
    return np.ascontiguousarray(xcols.reshape(xcols.shape[0], ND, 128).transpose(2, 1, 0))


def _routing(xf, router_w):
    """Replicate the reference's router math on CPU via jax (bit-matching ops).

    Returns (topk_idx [T, K] int, cw [T, E] f32, aux_loss f32 scalar).
    """
    import jax
    import jax.numpy as jnp
    cpu = jax.devices('cpu')[0]
    with jax.default_device(cpu):
        xj = jnp.asarray(xf)
        rwj = jnp.asarray(router_w)
        logits = xj @ rwj.T
        probs = jax.nn.softmax(logits, axis=-1)
        _, topk_idx = jax.lax.top_k(logits, TOPK)
        topk_p, _ = jax.lax.top_k(probs, TOPK)
        density = jax.nn.one_hot(topk_idx[:, 0], E, dtype=jnp.float32).mean(0)
        aux_loss = AUX_COEF * jnp.sum(density * probs.mean(0)) * E
        topk_idx = np.asarray(topk_idx)
        topk_p = np.asarray(topk_p)
        aux_loss = np.asarray(aux_loss)
    cwf = np.zeros((xf.shape[0], E), np.float32)
    np.add.at(cwf, (np.arange(xf.shape[0])[:, None], topk_idx), topk_p)
    return topk_idx, cwf, aux_loss


def kernel(x, router_w, w1, w2, w3, sw1, sw2, sw3, _run_opts=None):
    x = np.asarray(x, dtype=np.float32)
    router_w = np.asarray(router_w, dtype=np.float32)
    w1 = np.asarray(w1, dtype=np.float32)
    w2 = np.asarray(w2, dtype=np.float32)
    w3 = np.asarray(w3, dtype=np.float32)
    sw1 = np.asarray(sw1, dtype=np.float32)
    sw2 = np.asarray(sw2, dtype=np.float32)
    sw3 = np.asarray(sw3, dtype=np.float32)

    xf = x.reshape(T, D)
    topk_idx, cwf, aux_loss = _routing(xf, router_w)

    idx = [np.nonzero((topk_idx == e).any(axis=1))[0] for e in range(E)]
    counts = np.array([len(i) for i in idx])
    CA = max(256, int(-(-counts.max() // 256)) * 256)
    CTOT = CA + CB

    nc = _get_kernel(CA)

    w13B = _pack_w13(sw1[0], sw3[0])
    w2B = _pack_w2(sw2[0])

    in_maps = []
    for e in range(E):
        xcols = np.zeros((CTOT, D), np.float32)
        xcols[:counts[e]] = xf[idx[e]]
        xcols[CA:] = xf[e * CB:(e + 1) * CB]
        cw_col = np.zeros((CTOT, 1), np.float32)
        cw_col[:counts[e], 0] = cwf[idx[e], e]
        cw_col[CA:, 0] = 1.0
        in_maps.append({
            "xt": _pack_xt(xcols),
            "w13A": _pack_w13(w1[e], w3[e]),
            "w2A": _pack_w2(w2[e]),
            "w13B": w13B,
            "w2B": w2B,
            "cw": cw_col,
        })

    run_opts = _run_opts or {}
    res = run_bass_kernel_spmd(nc, in_maps, core_ids=list(range(NCORES)),
                               **run_opts)

    out = np.zeros((T, D), np.float32)
    for e in range(E):
        ye = res.results[e]["y"]
        out[idx[e]] += ye[:counts[e]]
        out[e * CB:(e + 1) * CB] += ye[CA:]

    if run_opts:
        kernel._last_result = res
    return out.reshape(B, S, D), aux_loss


# revision 2
# speedup vs baseline: 1.0571x; 1.0571x over previous
"""Trainium2 Bass kernel for nn_FFNwMoE (MoE FFN with top-2 routing + shared expert).

Strategy (expert-parallel sparse dispatch, host-side routing):
  - Host computes router logits/softmax/top-2 (jax on CPU, bit-matching the
    reference) plus the aux load-balancing loss.
  - Tokens are gathered per expert on the host. Core e processes expert e's
    tokens (padded to capacity CA) with expert-e weights, plus a static 1/8
    slice of all tokens (CB=1024) with the shared-expert weights.
  - On-device per core: swiglu via fp32r matmuls (full PE rate, ~FP22
    precision): aT/bT = W1/W3 contraction over d, h = silu(a)*b,
    y = hT.T @ W2T accumulated over h-tiles, scaled by the combine weight.
  - Host scatter-adds the per-core outputs back into the full [T, D] output.

All heavy FLOPs (3 matmuls x (2*T top-2 assignments + T shared)) run on the
8 NeuronCores; the host only does O(T*E) routing math and data movement.
All DRAM inputs are host-pre-tiled so DMA descriptors are >=16KB-contiguous
per partition.
"""
import sys

if '/opt/trn_rl_repo' not in sys.path:
    sys.path.insert(0, '/opt/trn_rl_repo')

from contextlib import ExitStack

import numpy as np

import concourse.bass as bass  # noqa: F401  (bass types used via tile/bacc)
import concourse.mybir as mybir
import concourse.tile as tile
from concourse import bacc
from concourse.bass_utils import run_bass_kernel_spmd

F32R = mybir.dt.float32r
F32 = mybir.dt.float32
AF = mybir.ActivationFunctionType

# Problem constants (hardcoded per spec nn_FFNwMoE_74380243632567)
B, S, D = 4, 2048, 2048
E, TOPK, H, SHARED = 8, 2, 1368, 1
AUX_COEF = 0.01
T = B * S                      # 8192 tokens
ND = D // 128                  # 16 d-tiles
NH = (H + 127) // 128          # 11 h-tiles (H padded 1368 -> 1408)
HPAD = NH * 128
NDC = D // 512                 # 4 output d-chunks
NCORES = 8
CB = T // NCORES               # shared-expert slice per core
TBMAX = 1024


def _block_list(CA, CB):
    blocks = []
    for ph, (start, size) in enumerate(((0, CA), (CA, CB))):
        off = start
        while off < start + size:
            TB = min(TBMAX, start + size - off)
            blocks.append((off, TB, ph))
            off += TB
    return blocks


def _build_moe_kernel(CA, CB):
    """One SPMD Bass program; per-core data arrives via in_maps."""
    CTOT = CA + CB
    NTTG = CTOT // 128
    nc = bacc.Bacc("TRN2", target_bir_lowering=False, debug=False,
                   num_devices=NCORES)

    # xt: block-contiguous flat layout; per block [128, ND, TB] with
    # partition-major contiguity (per-partition run = ND*TB*4 bytes).
    xt = nc.dram_tensor("xt", [128 * ND * CTOT], F32R, kind="ExternalInput").ap()
    w13A = nc.dram_tensor("w13A", [NH, 128, ND, 256], F32R, kind="ExternalInput").ap()
    w2A = nc.dram_tensor("w2A", [NDC, 128, NH, 512], F32R, kind="ExternalInput").ap()
    w13B = nc.dram_tensor("w13B", [NH, 128, ND, 256], F32R, kind="ExternalInput").ap()
    w2B = nc.dram_tensor("w2B", [NDC, 128, NH, 512], F32R, kind="ExternalInput").ap()
    cw = nc.dram_tensor("cw", [128, NTTG], F32, kind="ExternalInput").ap()
    y = nc.dram_tensor("y", [CTOT, D], F32, kind="ExternalOutput").ap()

    blocks = _block_list(CA, CB)

    with tile.TileContext(nc) as tc, ExitStack() as ctx:
        xt_pool = ctx.enter_context(tc.tile_pool(name="xtp", bufs=1))
        h_pool = ctx.enter_context(tc.tile_pool(name="hp", bufs=1))
        w13_pool = ctx.enter_context(tc.tile_pool(name="w13p", bufs=2))
        w2_pool = ctx.enter_context(tc.tile_pool(name="w2p", bufs=2))
        sil_pool = ctx.enter_context(tc.tile_pool(name="silp", bufs=2))
        y_pool = ctx.enter_context(tc.tile_pool(name="yp", bufs=3))
        cw_pool = ctx.enter_context(tc.tile_pool(name="cwp", bufs=1))
        psa_pool = ctx.enter_context(tc.tile_pool(name="psa", bufs=3, space="PSUM"))
        psb_pool = ctx.enter_context(tc.tile_pool(name="psb", bufs=3, space="PSUM"))
        psy_pool = ctx.enter_context(tc.tile_pool(name="psy", bufs=2, space="PSUM"))

        cw_all = cw_pool.tile([128, NTTG], F32, tag="cw")
        nc.sync.dma_start(cw_all[:], cw)

        for (t0, TB, ph) in blocks:
            w13X = w13A if ph == 0 else w13B
            w2X = w2A if ph == 0 else w2B
            ntt = TB // 128
            subs = []
            off = 0
            while off < TB:
                subs.append((off, min(512, TB - off)))
                off += 512

            xt_sb = xt_pool.tile([128, ND, TBMAX], F32R, tag="xt")
            xt_blk = xt[128 * ND * t0:128 * ND * (t0 + TB)].rearrange(
                "(p kd t) -> p kd t", p=128, kd=ND)
            half = ND // 2
            nc.sync.dma_start(xt_sb[:, :half, :TB], xt_blk[:, :half, :])
            nc.sync.dma_start(xt_sb[:, half:, :TB], xt_blk[:, half:, :])

            h_sb = h_pool.tile([128, NH, TBMAX], F32R, tag="hsb")

            # phase 1: aT/bT[h, t] accumulation over d-tiles; h = silu(a)*b
            for h in range(NH):
                w13_sb = w13_pool.tile([128, ND, 256], F32R, tag="w13")
                nc.sync.dma_start(w13_sb[:, :half, :], w13X[h, :, :half, :])
                nc.sync.dma_start(w13_sb[:, half:, :], w13X[h, :, half:, :])
                for (so, sw) in subs:
                    psa = psa_pool.tile([128, 512], F32, tag="psa")
                    psb = psb_pool.tile([128, 512], F32, tag="psb")
                    for kd in range(ND):
                        nc.tensor.matmul(psa[:, :sw], w13_sb[:, kd, 0:128],
                                         xt_sb[:, kd, so:so + sw],
                                         start=(kd == 0), stop=(kd == ND - 1))
                    for kd in range(ND):
                        nc.tensor.matmul(psb[:, :sw], w13_sb[:, kd, 128:256],
                                         xt_sb[:, kd, so:so + sw],
                                         start=(kd == 0), stop=(kd == ND - 1))
                    sil = sil_pool.tile([128, 512], F32, tag="sil")
                    nc.scalar.activation(sil[:, :sw], psa[:, :sw], AF.Sigmoid)
                    nc.vector.tensor_mul(sil[:, :sw], sil[:, :sw], psa[:, :sw])
                    nc.vector.tensor_mul(h_sb[:, h, so:so + sw], sil[:, :sw],
                                         psb[:, :sw])

            # phase 3: y[t, d] = hT.T @ w2T over h-tiles, scaled by cw[t]
            for dc in range(NDC):
                w2_sb = w2_pool.tile([128, NH, 512], F32R, tag="w2")
                hh = NH // 2
                nc.sync.dma_start(w2_sb[:, :hh, :], w2X[dc, :, :hh, :])
                nc.sync.dma_start(w2_sb[:, hh:, :], w2X[dc, :, hh:, :])
                for tt in range(ntt):
                    psy = psy_pool.tile([128, 512], F32, tag="psy")
                    for h in range(NH):
                        nc.tensor.matmul(psy[:],
                                         h_sb[:, h, tt * 128:(tt + 1) * 128],
                                         w2_sb[:, h, :],
                                         start=(h == 0), stop=(h == NH - 1))
                    y_sb = y_pool.tile([128, 512], F32, tag="y")
                    ttg = t0 // 128 + tt
                    nc.vector.tensor_scalar_mul(y_sb[:], psy[:],
                                                cw_all[:, ttg:ttg + 1])
                    nc.sync.dma_start(
                        y[t0 + tt * 128:t0 + (tt + 1) * 128,
                          dc * 512:(dc + 1) * 512], y_sb[:])

    nc.compile()
    return nc


_KERNEL_CACHE = {}


def _get_kernel(CA):
    key = (CA, CB)
    if key not in _KERNEL_CACHE:
        _KERNEL_CACHE[key] = _build_moe_kernel(CA, CB)
    return _KERNEL_CACHE[key]


# ---------------- host-side packing ----------------

def _pack_w13(w1, w3):
    """w1, w3: [H, D] -> [NH, 128, ND, 256] with w13[h,p,kd,j]=w1p[h*128+j, kd*128+p]."""
    w1p = np.zeros((HPAD, D), np.float32)
    w1p[:H] = w1
    w3p = np.zeros((HPAD, D), np.float32)
    w3p[:H] = w3
    a = w1p.reshape(NH, 128, ND, 128).transpose(0, 3, 2, 1)
    b = w3p.reshape(NH, 128, ND, 128).transpose(0, 3, 2, 1)
    return np.ascontiguousarray(np.concatenate([a, b], axis=3))


def _pack_w2(w2):
    """w2: [D, H] -> [NDC, 128, NH, 512] with w2t[dc,p,h,j] = w2[dc*512+j, h*128+p]."""
    w2tp = np.zeros((HPAD, D), np.float32)
    w2tp[:H] = w2.T
    return np.ascontiguousarray(
        w2tp.reshape(NH, 128, NDC, 512).transpose(2, 1, 0, 3))


def _pack_xt(xcols, CA):
    """xcols: [CTOT, D] -> block-contiguous flat [128*ND*CTOT]:
    per block, layout [p, kd, t_local] with xt[p,kd,t]=xcols[t, kd*128+p]."""
    CTOT = xcols.shape[0]
    out = np.empty(128 * ND * CTOT, np.float32)
    pos = 0
    for (t0, TB, _ph) in _block_list(CA, CTOT - CA):
        blk = xcols[t0:t0 + TB].reshape(TB, ND, 128).transpose(2, 1, 0)
        n = 128 * ND * TB
        out[pos:pos + n] = blk.reshape(-1)
        pos += n
    assert pos == out.size
    return out


def _routing(xf, router_w):
    """Replicate the reference's router math on CPU via jax (bit-matching ops).

    Returns (topk_idx [T, K] int, cw [T, E] f32, aux_loss f32 scalar).
    """
    import jax
    import jax.numpy as jnp
    cpu = jax.devices('cpu')[0]
    with jax.default_device(cpu):
        xj = jnp.asarray(xf)
        rwj = jnp.asarray(router_w)
        logits = xj @ rwj.T
        probs = jax.nn.softmax(logits, axis=-1)
        _, topk_idx = jax.lax.top_k(logits, TOPK)
        topk_p, _ = jax.lax.top_k(probs, TOPK)
        density = jax.nn.one_hot(topk_idx[:, 0], E, dtype=jnp.float32).mean(0)
        aux_loss = AUX_COEF * jnp.sum(density * probs.mean(0)) * E
        topk_idx = np.asarray(topk_idx)
        topk_p = np.asarray(topk_p)
        aux_loss = np.asarray(aux_loss)
    cwf = np.zeros((xf.shape[0], E), np.float32)
    np.add.at(cwf, (np.arange(xf.shape[0])[:, None], topk_idx), topk_p)
    return topk_idx, cwf, aux_loss


def kernel(x, router_w, w1, w2, w3, sw1, sw2, sw3, _run_opts=None):
    x = np.asarray(x, dtype=np.float32)
    router_w = np.asarray(router_w, dtype=np.float32)
    w1 = np.asarray(w1, dtype=np.float32)
    w2 = np.asarray(w2, dtype=np.float32)
    w3 = np.asarray(w3, dtype=np.float32)
    sw1 = np.asarray(sw1, dtype=np.float32)
    sw2 = np.asarray(sw2, dtype=np.float32)
    sw3 = np.asarray(sw3, dtype=np.float32)

    xf = x.reshape(T, D)
    topk_idx, cwf, aux_loss = _routing(xf, router_w)

    idx = [np.nonzero((topk_idx == e).any(axis=1))[0] for e in range(E)]
    counts = np.array([len(i) for i in idx])
    CA = max(256, int(-(-counts.max() // 256)) * 256)
    CTOT = CA + CB

    nc = _get_kernel(CA)

    w13B = _pack_w13(sw1[0], sw3[0])
    w2B = _pack_w2(sw2[0])

    in_maps = []
    for e in range(E):
        xcols = np.zeros((CTOT, D), np.float32)
        xcols[:counts[e]] = xf[idx[e]]
        xcols[CA:] = xf[e * CB:(e + 1) * CB]
        cw_col = np.zeros(CTOT, np.float32)
        cw_col[:counts[e]] = cwf[idx[e], e]
        cw_col[CA:] = 1.0
        in_maps.append({
            "xt": _pack_xt(xcols, CA),
            "w13A": _pack_w13(w1[e], w3[e]),
            "w2A": _pack_w2(w2[e]),
            "w13B": w13B,
            "w2B": w2B,
            "cw": np.ascontiguousarray(cw_col.reshape(CTOT // 128, 128).T),
        })

    run_opts = _run_opts or {}
    res = run_bass_kernel_spmd(nc, in_maps, core_ids=list(range(NCORES)),
                               **run_opts)

    out = np.zeros((T, D), np.float32)
    for e in range(E):
        ye = res.results[e]["y"]
        out[idx[e]] += ye[:counts[e]]
        out[e * CB:(e + 1) * CB] += ye[CA:]

    if run_opts:
        kernel._last_result = res
    return out.reshape(B, S, D), aux_loss
